# revision 1
# baseline (speedup 1.0000x reference)
"""Tensor-parallel causal multi-head attention (RoPE) on 8 TRN2 NeuronCores.

Sharding: heads are split across the 8 cores (16 heads -> 2 heads/core).
wq/wk/wv are split column-wise (by output head), wo row-wise; hidden_states
is replicated.  Each core computes its 2 heads end-to-end (QKV projection,
RoPE, causal attention, output projection) and returns its additive partial
of the full output; the host sums the 8 partials.

Device-side layout choices (all matmuls contract over the partition dim):
  - X^T [HID, B*S] is produced on the host so projections need no on-device
    transposes.  Q and K are computed directly in transposed layout
    Q^T/K^T [d, s] (lhsT = W^T chunk, rhs = X^T chunk), V in normal layout
    [s, d] (lhsT = X^T chunk, rhs = W^T).
  - Scores are computed transposed: S^T[k, q] = (K^T chunk).T @ Q^T, so the
    exp'd probabilities P^T [k, q] feed the O^T = V.T @ P^T matmul directly
    with q as the 512-wide moving dim (full fp32r rate), no transposes.
  - softmax denominators l[q] = sum_k P^T[k, q] come from a ones-column
    matmul accumulated alongside O^T in PSUM; 1/l (fast DVE reciprocal)
    is broadcast across partitions on the idle GpSimd engine.
  - No max-subtraction: scores are O(1) for this problem so exp is safe.
  - RoPE's rotate_half is a partition swap done with two SBUF->SBUF DMAs;
    the sign flip is folded into the host-prepared sin^T (lower half
    negated), and the 1/sqrt(D) score scale is folded into wq.
"""

import math

import numpy as np

import concourse.bass as bass
import concourse.tile as tile
from concourse import bacc, mybir
from concourse.bass_utils import run_bass_kernel_spmd

B, S, HID = 2, 2048, 2048
H, D = 16, 128
NCORES = 8
HPC = H // NCORES  # heads per core
DH = HPC * D  # per-core projection width (256)
NHC = HID // 128  # hid chunks (16)
TS = 512  # s-tile for projections
TQ = 512  # q-tile for attention
NKB = S // 128  # k blocks per sequence (16)
F32 = mybir.dt.float32
F32R = mybir.dt.float32r

LAST_EXEC_TIME_NS = None
_CACHE = {}


def _build_device_program():
    nc = bacc.Bacc(
        "TRN2",
        target_bir_lowering=False,
        debug=False,
        enable_asserts=False,
        num_devices=NCORES,
    )
    xT = nc.dram_tensor("xT", [HID, B * S], F32R, kind="ExternalInput").ap()
    wqT = nc.dram_tensor("wqT", [HID, DH], F32R, kind="ExternalInput").ap()
    wkT = nc.dram_tensor("wkT", [HID, DH], F32R, kind="ExternalInput").ap()
    wvT = nc.dram_tensor("wvT", [HID, DH], F32R, kind="ExternalInput").ap()
    woT = nc.dram_tensor("woT", [DH, HID], F32R, kind="ExternalInput").ap()
    cosT = nc.dram_tensor("cosT", [D, B * S], F32, kind="ExternalInput").ap()
    sinT = nc.dram_tensor("sinT", [D, B * S], F32, kind="ExternalInput").ap()
    out = nc.dram_tensor("out", [B * S, HID], F32, kind="ExternalOutput").ap()

    with tile.TileContext(nc) as tc:
        _emit_kernel(tc, xT, wqT, wkT, wvT, woT, cosT, sinT, out)

    nc.compile()
    return nc


def _emit_kernel(tc, xT, wqT, wkT, wvT, woT, cosT, sinT, out, dbg=None):
    from contextlib import ExitStack

    nc = tc.nc
    with ExitStack() as ctx:
        xTr = xT.rearrange("(hc p) s -> p hc s", p=128)  # [128, 16, B*S]
        wqTr = wqT.rearrange("(hc p) d -> p hc d", p=128)  # [128, 16, DH]
        wkTr = wkT.rearrange("(hc p) d -> p hc d", p=128)
        wvTr = wvT.rearrange("(hc p) d -> p hc d", p=128)
        woTr = woT.rearrange("(wc p) e -> p wc e", p=128)  # [128, HPC, HID]

        const = ctx.enter_context(tc.tile_pool(name="const", bufs=1))
        batchp = ctx.enter_context(tc.tile_pool(name="batchp", bufs=1))
        xtp = ctx.enter_context(tc.tile_pool(name="xtp", bufs=2))
        csp = ctx.enter_context(tc.tile_pool(name="csp", bufs=2))
        tmpp = ctx.enter_context(tc.tile_pool(name="tmpp", bufs=8))
        ptp = ctx.enter_context(tc.tile_pool(name="ptp", bufs=4))
        recp = ctx.enter_context(tc.tile_pool(name="recp", bufs=2))
        psump = ctx.enter_context(tc.tile_pool(name="psump", bufs=8, space="PSUM"))

        # ---- resident constants ----
        # split weight loads per hid-chunk group so the first matmuls only
        # wait for the chunks they read; wo is loaded later (phase C)
        wq_sb = const.tile([128, NHC, DH], F32R)
        wk_sb = const.tile([128, NHC, DH], F32R)
        wv_sb = const.tile([128, NHC, DH], F32R)
        for j in range(8):
            c0, c1 = j * 2, j * 2 + 2
            nc.scalar.dma_start(out=wq_sb[:, c0:c1, :], in_=wqTr[:, c0:c1, :])
            nc.scalar.dma_start(out=wk_sb[:, c0:c1, :], in_=wkTr[:, c0:c1, :])
            nc.scalar.dma_start(out=wv_sb[:, c0:c1, :], in_=wvTr[:, c0:c1, :])
        wo_sb = const.tile([128, HPC, HID], F32R)
        ones_f = const.tile([128, 1], F32)
        nc.vector.memset(ones_f[:], 1.0)
        ones_col = const.tile([128, 1], F32R)
        nc.scalar.copy(ones_col[:], ones_f[:])

        for b in range(B):
            bs = b * S
            # per-batch on-chip tensors (slots shared across batches via tags)
            qt_sb = batchp.tile([128, HPC, S], F32R, tag="qt")  # Q^T (scaled, roped)
            kt_sb = batchp.tile([128, HPC, S], F32R, tag="kt")  # K^T (roped)
            v_sb = batchp.tile([128, NKB * DH], F32R, tag="v")  # V row-blocks
            at_sb = batchp.tile([128, HPC, S], F32R, tag="at")  # attn out (A^T)

            # ---- phase A: QKV projections + RoPE ----
            for st in range(S // TS):
                s0 = st * TS
                psq = [
                    psump.tile([128, TS], F32, tag="big", name=f"psq{h}")
                    for h in range(HPC)
                ]
                psk = [
                    psump.tile([128, TS], F32, tag="big", name=f"psk{h}")
                    for h in range(HPC)
                ]
                psv = [
                    psump.tile([128, TS], F32, tag="big", name=f"psv{sp}")
                    for sp in range(TS // 256)
                ]
                for half in range(2):
                    xt = xtp.tile([128, 8, TS], F32R)
                    for xj in range(2):
                        nc.sync.dma_start(
                            out=xt[:, xj * 4 : xj * 4 + 4, :],
                            in_=xTr[
                                :,
                                half * 8 + xj * 4 : half * 8 + xj * 4 + 4,
                                bs + s0 : bs + s0 + TS,
                            ],
                        )
                    for i in range(8):
                        hc = half * 8 + i
                        first = hc == 0
                        last = hc == NHC - 1
                        for h in range(HPC):
                            nc.tensor.matmul(
                                psq[h][:],
                                lhsT=(wq_sb[:, hc, h * D : (h + 1) * D]),
                                rhs=(xt[:, i, :]),
                                start=first,
                                stop=last,
                            )
                            nc.tensor.matmul(
                                psk[h][:],
                                lhsT=(wk_sb[:, hc, h * D : (h + 1) * D]),
                                rhs=(xt[:, i, :]),
                                start=first,
                                stop=last,
                            )
                        for sp in range(TS // 256):
                            for sblk in range(2):
                                # one accumulation group per PSUM bank:
                                # start=True clears the whole bank, so only
                                # the first matmul touching the tile starts
                                nc.tensor.matmul(
                                    psv[sp][:, sblk * DH : (sblk + 1) * DH],
                                    lhsT=(
                                        xt[:, i, (sp * 2 + sblk) * 128 : (sp * 2 + sblk + 1) * 128]
                                    ),
                                    rhs=(wv_sb[:, hc, :]),
                                    start=first and sblk == 0,
                                    stop=last and sblk == 1,
                                    skip_group_check=True,
                                )
                # V: evacuate PSUM -> v_sb
                for sp in range(TS // 256):
                    blk0 = s0 // 128 + sp * 2
                    nc.scalar.copy(
                        v_sb[:, blk0 * DH : (blk0 + 2) * DH], psv[sp][:]
                    )
                # RoPE for Q and K
                cs = csp.tile([128, TS], F32, tag="cs")
                nc.sync.dma_start(out=cs[:], in_=cosT[:, bs + s0 : bs + s0 + TS])
                sn = csp.tile([128, TS], F32, tag="cs")
                nc.sync.dma_start(out=sn[:], in_=sinT[:, bs + s0 : bs + s0 + TS])
                for ps_list, dst in ((psq, qt_sb), (psk, kt_sb)):
                    for h in range(HPC):
                        ps = ps_list[h]
                        tq = tmpp.tile([128, TS], F32, tag="tmp")
                        nc.scalar.copy(tq[:], ps[:])
                        tc_cos = tmpp.tile([128, TS], F32, tag="tmp")
                        nc.vector.tensor_mul(tc_cos[:], ps[:], cs[:])
                        tqs = tmpp.tile([128, TS], F32, tag="tmp")
                        nc.sync.dma_start(out=tqs[0:64, :], in_=tq[64:128, :])
                        nc.sync.dma_start(out=tqs[64:128, :], in_=tq[0:64, :])
                        nc.vector.tensor_mul(tqs[:], tqs[:], sn[:])
                        nc.vector.tensor_add(
                            dst[:, h, s0 : s0 + TS], tc_cos[:], tqs[:]
                        )

            if dbg is not None and b == 0:
                nc.sync.dma_start(out=dbg["dqt"][:], in_=qt_sb[:].bitcast(F32))
                nc.sync.dma_start(out=dbg["dkt"][:], in_=kt_sb[:].bitcast(F32))
                nc.sync.dma_start(out=dbg["dv"][:], in_=v_sb[:].bitcast(F32))

            # ---- phase B: causal attention ----
            if b == 0:
                nc.scalar.dma_start(out=wo_sb[:], in_=woTr[:])
            for qt in range(S // TQ):
                q0 = qt * TQ
                for h in range(HPC):
                    nvis = (q0 + TQ) // 128
                    pso = psump.tile([128, TQ], F32, tag="big")
                    psl = psump.tile([1, TQ], F32, tag="big")

                    def score_block(kb):
                        # trim the moving dim to the causal region (min 256
                        # wide so fp32r stays at full rate)
                        off = max(0, kb * 128 - q0)
                        off = min(off, TQ - 256)
                        W = TQ - off
                        pss = psump.tile([128, TQ], F32, tag="big", name="pss")
                        nc.tensor.matmul(
                            pss[:, 0:W],
                            lhsT=(kt_sb[:, h, kb * 128 : (kb + 1) * 128]),
                            rhs=(qt_sb[:, h, q0 + off : q0 + TQ]),
                            start=True,
                            stop=True,
                        )
                        pt = ptp.tile([128, TQ], F32R, tag="pt", name="pt")
                        nc.scalar.activation(
                            pt[:, 0:W],
                            pss[:, 0:W],
                            func=mybir.ActivationFunctionType.Exp,
                        )
                        if kb * 128 + 127 > q0:
                            # diagonal block: zero future positions
                            nc.gpsimd.affine_select(
                                out=pt[:, 0:W],
                                in_=pt[:, 0:W],
                                pattern=[[1, W]],
                                base=q0 + off - kb * 128,
                                channel_multiplier=-1,
                                compare_op=mybir.AluOpType.is_ge,
                                fill=0.0,
                            )
                        return pt, off, W

                    def av_block(kb, pt, off, W):
                        first = kb == 0
                        last = kb == nvis - 1
                        nc.tensor.matmul(
                            pso[:, off:TQ],
                            lhsT=(v_sb[:, kb * DH + h * D : kb * DH + (h + 1) * D]),
                            rhs=(pt[:, 0:W]),
                            start=first,
                            stop=last,
                            skip_group_check=True,
                        )
                        nc.tensor.matmul(
                            psl[:, off:TQ],
                            lhsT=(ones_col[:]),
                            rhs=(pt[:, 0:W]),
                            start=first,
                            stop=last,
                            skip_group_check=True,
                        )
                        if dbg is not None and b == 0 and h == 0 and qt == 3:
                            nc.sync.dma_start(
                                out=dbg["dpt"][:, kb, 0:W], in_=pt[:, 0:W].bitcast(F32)
                            )
                            if off:
                                nc.gpsimd.memset(dbg["dpt"][:, kb, W:TQ], 0.0)

                    # software pipeline: scores run one k-block ahead of AV
                    # so the exp latency is hidden behind PE work
                    pending = None
                    for kb in range(nvis):
                        cur = (kb, *score_block(kb))
                        if pending is not None:
                            av_block(*pending)
                        pending = cur
                    av_block(*pending)
                    if dbg is not None and b == 0 and h == 0:
                        lrow = tmpp.tile([1, TQ], F32, tag="lrow", bufs=1)
                        nc.vector.tensor_copy(lrow[:], psl[:])
                        nc.sync.dma_start(out=dbg["dl"][:, q0 : q0 + TQ], in_=lrow[:])
                    rec = recp.tile([1, TQ], F32, tag="rec")
                    nc.vector.reciprocal_approx_fast(out=rec[:], in_=psl[:])
                    rb = tmpp.tile([128, TQ], F32, tag="tmp")
                    nc.gpsimd.partition_broadcast(rb[:], rec[:])
                    nc.vector.tensor_mul(at_sb[:, h, q0 : q0 + TQ], pso[:], rb[:])

            if dbg is not None and b == 0:
                nc.sync.dma_start(out=dbg["dat"][:], in_=at_sb[:].bitcast(F32))

            # ---- phase C: output projection (partial over local heads) ----
            for sb in range(S // 128):
                for ep in range(2):
                    psus = [
                        psump.tile([128, 512], F32, tag="big", name=f"psu{eu}")
                        for eu in range(2)
                    ]
                    for h in range(HPC):
                        for eu in range(2):
                            et = ep * 2 + eu
                            nc.tensor.matmul(
                                psus[eu][:],
                                lhsT=(at_sb[:, h, sb * 128 : (sb + 1) * 128]),
                                rhs=(wo_sb[:, h, et * 512 : (et + 1) * 512]),
                                start=h == 0,
                                stop=h == HPC - 1,
                            )
                    for eu in range(2):
                        et = ep * 2 + eu
                        ub = tmpp.tile([128, 512], F32, tag="tmp")
                        nc.scalar.copy(ub[:, 0:256], psus[eu][:, 0:256])
                        nc.vector.tensor_copy(ub[:, 256:512], psus[eu][:, 256:512])
                        nc.sync.dma_start(
                            out=out[
                                bs + sb * 128 : bs + (sb + 1) * 128,
                                et * 512 : (et + 1) * 512,
                            ],
                            in_=ub[:],
                        )


def _host_inputs(hidden_states, cos, sin, wq, wk, wv, wo):
    x = np.ascontiguousarray(np.asarray(hidden_states, dtype=np.float32)).reshape(
        B * S, HID
    )
    xT = np.ascontiguousarray(x.T)
    cos = np.asarray(cos, dtype=np.float32)
    sin = np.asarray(sin, dtype=np.float32)
    # [D, B*S], column b*S+s = cos[b, s, :]
    cosT = np.ascontiguousarray(cos.reshape(B * S, D).T)
    sinT = np.ascontiguousarray(sin.reshape(B * S, D).T)
    sinT[: D // 2, :] *= -1.0  # fold rotate_half's negation into sin
    wq = np.asarray(wq, dtype=np.float32)
    wk = np.asarray(wk, dtype=np.float32)
    wv = np.asarray(wv, dtype=np.float32)
    wo = np.asarray(wo, dtype=np.float32)
    scale = 1.0 / math.sqrt(D)
    in_maps = []
    for c in range(NCORES):
        sl = slice(c * DH, (c + 1) * DH)
        in_maps.append(
            {
                "xT": xT,
                "wqT": np.ascontiguousarray(wq[sl].T * scale),
                "wkT": np.ascontiguousarray(wk[sl].T),
                "wvT": np.ascontiguousarray(wv[sl].T),
                "woT": np.ascontiguousarray(wo[:, sl].T),
                "cosT": cosT,
                "sinT": sinT,
            }
        )
    return in_maps


def kernel(
    hidden_states,
    cos,
    sin,
    wq,
    wk,
    wv,
    wo,
    position_ids=None,
    _trace=False,
    _tmpdir=None,
):
    global LAST_EXEC_TIME_NS
    if "nc" not in _CACHE:
        _CACHE["nc"] = _build_device_program()
    nc = _CACHE["nc"]
    in_maps = _host_inputs(hidden_states, cos, sin, wq, wk, wv, wo)
    res = run_bass_kernel_spmd(
        nc,
        in_maps,
        list(range(NCORES)),
        trace=_trace,
        tmpdir=_tmpdir,
    )
    LAST_EXEC_TIME_NS = res.exec_time_ns
    total = res.results[0]["out"].astype(np.float64)
    for c in range(1, NCORES):
        total += res.results[c]["out"]
    return total.astype(np.float32).reshape(B, S, HID)



# revision 9
# speedup vs baseline: 1.0262x; 1.0262x over previous
"""Tensor-parallel causal multi-head attention (RoPE) on 8 TRN2 NeuronCores.

Sharding: heads split across the 8 cores (16 heads -> 2 heads/core); wq/wk/wv
split column-wise, wo row-wise; hidden_states replicated.  Each core computes
its 2 heads end-to-end and writes a bf16 additive partial of the full output;
the host sums the 8 partials.

Key optimizations over the fp32r baseline (443us):
  - Everything runs in bf16 (same 1 cycle/row PE rate as fp32r but half the
    DMA/SBUF traffic and robust LDWEIGHTS hiding); fp32 PSUM accumulation
    throughout.  Measured end-to-end error vs the fp32 reference is ~3.5e-3
    (fp8 anywhere on the Q/K or value path breaks the 2e-2 tolerance).
  - RoPE's rotate_half is a single DVE stream_shuffle: the head_dim rows are
    host-permuted so each rotate pair sits 16 partitions apart within a
    32-partition block (stream_shuffle's reach), and the sign flip plus the
    shuffle-compensating permutation are folded into the host-prepared sin.
  - Causal masking is a DVE add of a precomputed -1e30 mask pair onto the
    score PSUM before exp (no gpsimd affine_select on the critical path).
  - exp runs once per k-block PAIR over a [128,2,W] PSUM view to halve the
    ~200-cycle ACT instruction overhead; q-tiles are 1024 wide to halve the
    softmax-normalization overhead per column.
  - Output partials are written as bf16 (half the write traffic; error is
    ~1e-4 of tolerance), one 4KB-per-partition DMA per 128-row block.
"""

import math

import numpy as np
import ml_dtypes

import concourse.bass as bass
import concourse.tile as tile
from concourse import bacc, mybir
from concourse.bass_utils import run_bass_kernel_spmd

B, S, HID = 2, 2048, 2048
H, D = 16, 128
NCORES = 8
HPC = H // NCORES  # heads per core (2)
DH = HPC * D  # per-core projection width (256)
NHC = HID // 128  # hid chunks (16)
NJ = NHC // 2  # hid chunk pairs (8)
TS = 512  # s-tile for projections
TQ = 1024  # q-tile for attention
NKB = S // 128  # k blocks per sequence (16)

ALPHA = 1.0 / math.sqrt(D)  # softmax scale, folded into exp()

F32 = mybir.dt.float32
BF16 = mybir.dt.bfloat16
NP_BF16 = ml_dtypes.bfloat16

# head-dim permutation: rotate pairs (j, j+64) are placed 16 apart within a
# 32-partition block so stream_shuffle(+16 mod 32) performs rotate_half
_dd = np.arange(128)
_b, _r = _dd // 32, _dd % 32
PERM = np.where(_r < 16, 16 * _b + _r, 64 + 16 * _b + (_r - 16))
PARTNER = (_dd // 32) * 32 + (_dd % 32 + 16) % 32
SGN = np.where(_dd % 32 < 16, -1.0, 1.0)  # rotate_half negates orig rows <64
SHUF_MASK = [(i + 16) % 32 for i in range(32)]

LAST_EXEC_TIME_NS = None
_CACHE = {}


def _build_device_program():
    nc = bacc.Bacc(
        "TRN2",
        target_bir_lowering=False,
        debug=False,
        enable_asserts=False,
        num_devices=NCORES,
    )
    xTb = nc.dram_tensor("xTb", [HID, B * S], BF16, kind="ExternalInput").ap()
    wqb = nc.dram_tensor("wqb", [128, NHC, DH], BF16, kind="ExternalInput").ap()
    wkb = nc.dram_tensor("wkb", [128, NHC, DH], BF16, kind="ExternalInput").ap()
    wvb = nc.dram_tensor("wvb", [128, NHC, DH], BF16, kind="ExternalInput").ap()
    wob = nc.dram_tensor("wob", [128, HPC, HID], BF16, kind="ExternalInput").ap()
    cosb = nc.dram_tensor("cosb", [128, B * S], BF16, kind="ExternalInput").ap()
    sinb = nc.dram_tensor("sinb", [128, B * S], BF16, kind="ExternalInput").ap()
    maskp = nc.dram_tensor("maskp", [128, 2, 512], BF16, kind="ExternalInput").ap()
    onesb = nc.dram_tensor("onesb", [128, 1], BF16, kind="ExternalInput").ap()
    out = nc.dram_tensor("out", [B * S, HID], BF16, kind="ExternalOutput").ap()

    with tile.TileContext(nc) as tc:
        _emit_kernel(tc, xTb, wqb, wkb, wvb, wob, cosb, sinb, maskp, onesb, out)

    nc.compile()
    return nc


def _emit_kernel(tc, xTb, wqb, wkb, wvb, wob, cosb, sinb, maskp, onesb, out):
    from contextlib import ExitStack

    nc = tc.nc
    xbr = xTb.rearrange("(hc p) s -> p hc s", p=128)  # [128, 16, B*S]
    outr = out.rearrange("s (g e) -> s g e", e=512)  # [B*S, 4, 512]

    with ExitStack() as ctx:
        const = ctx.enter_context(tc.tile_pool(name="const", bufs=1))
        batchp = ctx.enter_context(tc.tile_pool(name="batchp", bufs=2))
        xtp = ctx.enter_context(tc.tile_pool(name="xtp", bufs=2))
        csp = ctx.enter_context(tc.tile_pool(name="csp", bufs=2))
        tmpp = ctx.enter_context(tc.tile_pool(name="tmpp", bufs=2))
        ptp = ctx.enter_context(tc.tile_pool(name="ptp", bufs=3))
        ubp = ctx.enter_context(tc.tile_pool(name="ubp", bufs=2))
        psump = ctx.enter_context(tc.tile_pool(name="psump", bufs=4, space="PSUM"))

        # ---- resident constants ----
        wq_sb = const.tile([128, NHC, DH], BF16)
        wk_sb = const.tile([128, NHC, DH], BF16)
        wv_sb = const.tile([128, NHC, DH], BF16)
        wo_sb = const.tile([128, HPC, HID], BF16)
        mask_sb = const.tile([128, 2, 512], BF16)
        ones_sb = const.tile([128, 1], BF16)
        for j in range(4):
            nc.scalar.dma_start(out=wq_sb[:, j * 4 : j * 4 + 4], in_=wqb[:, j * 4 : j * 4 + 4])
            nc.scalar.dma_start(out=wk_sb[:, j * 4 : j * 4 + 4], in_=wkb[:, j * 4 : j * 4 + 4])
            nc.scalar.dma_start(out=wv_sb[:, j * 4 : j * 4 + 4], in_=wvb[:, j * 4 : j * 4 + 4])
        nc.scalar.dma_start(out=wo_sb[:], in_=wob)
        nc.scalar.dma_start(out=mask_sb[:], in_=maskp)
        nc.scalar.dma_start(out=ones_sb[:], in_=onesb)

        for b in range(B):
            bs = b * S
            qt_sb = batchp.tile([128, HPC, S], BF16, tag="qt")  # Q^T (x32, roped)
            kt_sb = batchp.tile([128, HPC, S], BF16, tag="kt")  # K^T (x32, roped)
            v_sb = batchp.tile([128, NKB, HPC, D], BF16, tag="v")  # V row-blocks
            at_sb = batchp.tile([128, HPC, S], BF16, tag="at")  # attn out (A^T)

            # ---- phase A: QKV projections + RoPE ----
            for st in range(S // TS):
                s0 = st * TS
                g0 = bs + s0
                xtb = xtp.tile([128, NHC, TS], BF16, tag="xb")
                for q4 in range(4):
                    nc.sync.dma_start(
                        out=xtb[:, q4 * 4 : q4 * 4 + 4, :],
                        in_=xbr[:, q4 * 4 : q4 * 4 + 4, g0 : g0 + TS],
                    )
                cs = csp.tile([128, TS], BF16, tag="cs")
                nc.sync.dma_start(out=cs[:], in_=cosb[:, g0 : g0 + TS])
                sn = csp.tile([128, TS], BF16, tag="sn")
                nc.sync.dma_start(out=sn[:], in_=sinb[:, g0 : g0 + TS])

                psq = psump.tile([128, 2, TS], F32, tag="ps", name="psq")
                psk = psump.tile([128, 2, TS], F32, tag="ps", name="psk")
                psv = psump.tile([128, 2, 2, HPC, D], F32, tag="ps", name="psv")
                for c in range(NHC):
                    first, last = c == 0, c == NHC - 1
                    for h in range(HPC):
                        nc.tensor.matmul(
                            psq[:, h, :],
                            lhsT=wq_sb[:, c, h * D : (h + 1) * D],
                            rhs=xtb[:, c, :],
                            start=first,
                            stop=last,
                        )
                        nc.tensor.matmul(
                            psk[:, h, :],
                            lhsT=wk_sb[:, c, h * D : (h + 1) * D],
                            rhs=xtb[:, c, :],
                            start=first,
                            stop=last,
                        )
                    for sbk in range(4):
                        sp, s2 = sbk // 2, sbk % 2
                        nc.tensor.matmul(
                            psv[:, sp, s2],
                            lhsT=xtb[:, c, sbk * 128 : (sbk + 1) * 128],
                            rhs=wv_sb[:, c, :],
                            start=first and s2 == 0,
                            stop=last and s2 == 1,
                            skip_group_check=True,
                        )
                # V: evacuate PSUM -> v_sb (bf16)
                for sp in range(2):
                    kb0 = st * 4 + 2 * sp
                    nc.scalar.copy(v_sb[:, kb0 : kb0 + 2, :, :], psv[:, sp])
                # RoPE on DVE (+gpsimd add): dst = ps*cos + shuffle(ps*sin_sw)
                for ps_t, dst in ((psq, qt_sb), (psk, kt_sb)):
                    for h in range(HPC):
                        ps = ps_t[:, h, :]
                        y = tmpp.tile([128, TS], BF16, tag="y")
                        nc.vector.tensor_mul(y[:], ps, sn[:])
                        ysw = tmpp.tile([128, TS], BF16, tag="ysw")
                        nc.vector.stream_shuffle(ysw[:], y[:], SHUF_MASK)
                        tcs = tmpp.tile([128, TS], BF16, tag="tc")
                        nc.vector.tensor_mul(tcs[:], ps, cs[:])
                        nc.gpsimd.tensor_add(dst[:, h, s0 : s0 + TS], tcs[:], ysw[:])

            # ---- phase B: causal attention ----
            for qt in range(S // TQ):
                q0 = qt * TQ
                npairs = (q0 + TQ) // 256
                # chunk list: (kb0, lo, wc, masked); halves never cross banks
                chunks = []
                for i in range(npairs):
                    kb0 = 2 * i
                    offf = min(max(kb0 * 128 - q0, 0), TQ - 256)
                    diag = kb0 * 128 >= q0
                    for hb in range(2):
                        lo = max(offf, hb * 512)
                        hi = (hb + 1) * 512
                        if lo < hi:
                            chunks.append((kb0, lo, hi - lo, diag and lo == offf))
                half_first = {}
                half_last = {}
                for idx, (kb0, lo, wc, m) in enumerate(chunks):
                    hb = lo // 512
                    if hb not in half_first:
                        half_first[hb] = idx
                    half_last[hb] = idx

                for h in range(HPC):
                    pso = psump.tile([128, TQ], F32, tag="ps", name="pso")
                    psl = psump.tile([1, TQ], F32, tag="ps", name="psl")

                    def score_chunk(idx):
                        kb0, lo, wc, masked = chunks[idx]
                        pss = psump.tile([128, 2, 512], F32, tag="ps", name="pss")
                        for t in range(2):
                            nc.tensor.matmul(
                                pss[:, t, 0:wc],
                                lhsT=kt_sb[:, h, (kb0 + t) * 128 : (kb0 + t + 1) * 128],
                                rhs=qt_sb[:, h, q0 + lo : q0 + lo + wc],
                                start=True,
                                stop=True,
                            )
                        if masked:
                            nc.vector.tensor_add(
                                pss[:, :, 0:wc], pss[:, :, 0:wc], mask_sb[:, :, 0:wc]
                            )
                        pt = ptp.tile([128, 2, 512], BF16, tag="pt")
                        nc.scalar.activation(
                            pt[:, :, 0:wc],
                            pss[:, :, 0:wc],
                            func=mybir.ActivationFunctionType.Exp,
                            scale=ALPHA,
                        )
                        return idx, pt

                    def av_chunk(idx, pt):
                        kb0, lo, wc, _ = chunks[idx]
                        hb = lo // 512
                        for t in range(2):
                            nc.tensor.matmul(
                                pso[:, lo : lo + wc],
                                lhsT=v_sb[:, kb0 + t, h, :],
                                rhs=pt[:, t, 0:wc],
                                start=half_first[hb] == idx and t == 0,
                                stop=half_last[hb] == idx and t == 1,
                                skip_group_check=True,
                            )
                            nc.tensor.matmul(
                                psl[:, lo : lo + wc],
                                lhsT=ones_sb[:],
                                rhs=pt[:, t, 0:wc],
                                start=half_first[hb] == idx and t == 0,
                                stop=half_last[hb] == idx and t == 1,
                                skip_group_check=True,
                            )

                    pending = None
                    for idx in range(len(chunks)):
                        cur = score_chunk(idx)
                        if pending is not None:
                            av_chunk(*pending)
                        pending = cur
                    av_chunk(*pending)

                    rec = tmpp.tile([1, TQ], F32, tag="rec")
                    nc.vector.reciprocal_approx_fast(out=rec[:], in_=psl[:])
                    rb = tmpp.tile([128, TQ], F32, tag="rb")
                    nc.gpsimd.partition_broadcast(rb[:], rec[:])
                    nc.vector.tensor_mul(at_sb[:, h, q0 : q0 + TQ], pso[:], rb[:])

            # ---- phase C: output projection (partial over local heads) ----
            for sb in range(S // 128):
                psu = [
                    psump.tile([128, 2, 512], F32, tag="ps", name=f"psu{ep}")
                    for ep in range(2)
                ]
                for et in range(4):
                    for h in range(HPC):
                        nc.tensor.matmul(
                            psu[et // 2][:, et % 2, :],
                            lhsT=at_sb[:, h, sb * 128 : (sb + 1) * 128],
                            rhs=wo_sb[:, h, et * 512 : (et + 1) * 512],
                            start=h == 0,
                            stop=h == HPC - 1,
                        )
                ub = ubp.tile([128, 4, 512], BF16, tag="ub")
                nc.scalar.copy(ub[:, 0:2, :], psu[0][:])
                nc.vector.tensor_copy(ub[:, 2:4, :], psu[1][:])
                nc.sync.dma_start(
                    out=outr[bs + sb * 128 : bs + (sb + 1) * 128, :, :], in_=ub[:]
                )


def _host_inputs(hidden_states, cos, sin, wq, wk, wv, wo):
    x = np.ascontiguousarray(np.asarray(hidden_states, dtype=np.float32)).reshape(
        B * S, HID
    )
    xTb = np.ascontiguousarray(x.T).astype(NP_BF16)
    cos = np.asarray(cos, dtype=np.float32).reshape(B * S, D)
    sin = np.asarray(sin, dtype=np.float32).reshape(B * S, D)
    cosb = np.ascontiguousarray(cos.T[PERM]).astype(NP_BF16)  # [128, B*S]
    snfold = SGN[:, None] * sin.T[PERM]
    sinb = np.ascontiguousarray(snfold[PARTNER]).astype(NP_BF16)
    wq = np.asarray(wq, dtype=np.float32)
    wk = np.asarray(wk, dtype=np.float32)
    wv = np.asarray(wv, dtype=np.float32)
    wo = np.asarray(wo, dtype=np.float32)

    jj = np.arange(512)[None, :]
    pp = np.arange(128)[:, None]
    maskp = np.empty((128, 2, 512), np.float32)
    maskp[:, 0, :] = np.where(jj >= pp, 0.0, -1e30)
    maskp[:, 1, :] = np.where(jj >= 128 + pp, 0.0, -1e30)
    maskp = maskp.astype(NP_BF16)
    onesb = np.ones((128, 1), NP_BF16)

    in_maps = []
    for c in range(NCORES):
        sl = slice(c * DH, (c + 1) * DH)
        # [h, dd(permuted), hid] -> [p, hc, (h dd)]
        wqp = wq[sl].reshape(HPC, 128, NHC, 128)[:, PERM, :, :]
        wkp = wk[sl].reshape(HPC, 128, NHC, 128)[:, PERM, :, :]
        wqb = np.ascontiguousarray(np.transpose(wqp, (3, 2, 0, 1))).reshape(
            128, NHC, DH
        ).astype(NP_BF16)
        wkb = np.ascontiguousarray(np.transpose(wkp, (3, 2, 0, 1))).reshape(
            128, NHC, DH
        ).astype(NP_BF16)
        # wvb[p, c, o] = wv_l[o, c*128+p]
        wvb = np.ascontiguousarray(
            np.transpose(wv[sl].reshape(DH, NHC, 128), (2, 1, 0))
        ).astype(NP_BF16)
        # wob[p, h, e] = wo[e, c0 + h*128 + p]
        wob = np.ascontiguousarray(
            np.transpose(wo[:, sl].reshape(HID, HPC, 128), (2, 1, 0))
        ).astype(NP_BF16)
        in_maps.append(
            {
                "xTb": xTb,
                "wqb": wqb,
                "wkb": wkb,
                "wvb": wvb,
                "wob": wob,
                "cosb": cosb,
                "sinb": sinb,
                "maskp": maskp,
                "onesb": onesb,
            }
        )
    return in_maps


def kernel(
    hidden_states,
    cos,
    sin,
    wq,
    wk,
    wv,
    wo,
    position_ids=None,
    _trace=False,
    _tmpdir=None,
):
    global LAST_EXEC_TIME_NS
    if "nc" not in _CACHE:
        _CACHE["nc"] = _build_device_program()
    nc = _CACHE["nc"]
    in_maps = _host_inputs(hidden_states, cos, sin, wq, wk, wv, wo)
    res = run_bass_kernel_spmd(
        nc,
        in_maps,
        list(range(NCORES)),
        trace=_trace,
        tmpdir=_tmpdir,
    )
    LAST_EXEC_TIME_NS = res.exec_time_ns
    total = res.results[0]["out"].astype(np.float64)
    for c in range(1, NCORES):
        total += res.results[c]["out"].astype(np.float64)
    return total.astype(np.float32).reshape(B, S, HID)


# revision 19
# speedup vs baseline: 1.1399x; 1.1108x over previous
"""Tensor-parallel causal multi-head attention (RoPE) on 8 TRN2 NeuronCores.

Sharding: heads split across the 8 cores (16 heads -> 2 heads/core); wq/wk/wv
split column-wise, wo row-wise; hidden_states replicated.  Each core computes
its 2 heads end-to-end and writes a bf16 additive partial of the full output;
the host sums the 8 partials.

Key optimizations over the fp32r baseline (443us):
  - Everything runs in bf16 (same 1 cycle/row PE rate as fp32r but half the
    DMA/SBUF traffic and robust LDWEIGHTS hiding); fp32 PSUM accumulation
    throughout.  Measured end-to-end error vs the fp32 reference is ~3.5e-3
    (fp8 anywhere on the Q/K or value path breaks the 2e-2 tolerance).
  - RoPE's rotate_half is a single DVE stream_shuffle: the head_dim rows are
    host-permuted so each rotate pair sits 16 partitions apart within a
    32-partition block (stream_shuffle's reach), and the sign flip plus the
    shuffle-compensating permutation are folded into the host-prepared sin.
  - Causal masking is a DVE add of a precomputed -1e30 mask pair onto the
    score PSUM before exp (no gpsimd affine_select on the critical path).
  - exp runs once per k-block PAIR over a [128,2,W] PSUM view to halve the
    ~200-cycle ACT instruction overhead; q-tiles are 1024 wide to halve the
    softmax-normalization overhead per column.
  - Output partials are written as bf16 (half the write traffic; error is
    ~1e-4 of tolerance), one 4KB-per-partition DMA per 128-row block.
"""

import math

import numpy as np
import ml_dtypes

import concourse.bass as bass
import concourse.tile as tile
from concourse import bacc, mybir
from concourse.bass_utils import run_bass_kernel_spmd

B, S, HID = 2, 2048, 2048
H, D = 16, 128
NCORES = 8
HPC = H // NCORES  # heads per core (2)
DH = HPC * D  # per-core projection width (256)
NHC = HID // 128  # hid chunks (16)
NJ = NHC // 2  # hid chunk pairs (8)
TS = 512  # s-tile for projections
TQ = 1024  # q-tile for attention
NKB = S // 128  # k blocks per sequence (16)

Q_FP8 = True  # Q projection in fp8 DoubleRow (K/V stay bf16)
SW = 32.0  # fp8 weight/activation scale for the Q path
ALPHA = (1.0 / (SW * math.sqrt(D))) if Q_FP8 else (1.0 / math.sqrt(D))

F32 = mybir.dt.float32
BF16 = mybir.dt.bfloat16
FP8 = mybir.dt.float8e4
DR = mybir.MatmulPerfMode.DoubleRow
NP_BF16 = ml_dtypes.bfloat16
NP_FP8 = ml_dtypes.float8_e4m3

# head-dim permutation: rotate pairs (j, j+64) are placed 16 apart within a
# 32-partition block so stream_shuffle(+16 mod 32) performs rotate_half
_dd = np.arange(128)
_b, _r = _dd // 32, _dd % 32
PERM = np.where(_r < 16, 16 * _b + _r, 64 + 16 * _b + (_r - 16))
PARTNER = (_dd // 32) * 32 + (_dd % 32 + 16) % 32
SGN = np.where(_dd % 32 < 16, -1.0, 1.0)  # rotate_half negates orig rows <64
SHUF_MASK = [(i + 16) % 32 for i in range(32)]

LAST_EXEC_TIME_NS = None
_CACHE = {}


def _build_device_program():
    nc = bacc.Bacc(
        "TRN2",
        target_bir_lowering=False,
        debug=False,
        enable_asserts=False,
        num_devices=NCORES,
    )
    xTb = nc.dram_tensor("xTb", [HID, B * S], BF16, kind="ExternalInput").ap()
    xT8 = nc.dram_tensor("xT8", [HID, B * S], FP8, kind="ExternalInput").ap()
    wq8 = nc.dram_tensor("wq8", [128, NJ, 2, DH], FP8, kind="ExternalInput").ap()
    wkb = nc.dram_tensor("wkb", [128, NHC, DH], BF16, kind="ExternalInput").ap()
    wvb = nc.dram_tensor("wvb", [128, NHC, DH], BF16, kind="ExternalInput").ap()
    wob = nc.dram_tensor("wob", [128, HPC, HID], BF16, kind="ExternalInput").ap()
    cosb = nc.dram_tensor("cosb", [128, B * S], BF16, kind="ExternalInput").ap()
    sinb = nc.dram_tensor("sinb", [128, B * S], BF16, kind="ExternalInput").ap()
    maskp = nc.dram_tensor("maskp", [128, 2, 512], BF16, kind="ExternalInput").ap()
    onesb = nc.dram_tensor("onesb", [128, 1], BF16, kind="ExternalInput").ap()
    out = nc.dram_tensor("out", [B * S, HID], BF16, kind="ExternalOutput").ap()

    with tile.TileContext(nc) as tc:
        _emit_kernel(tc, xTb, xT8, wq8, wkb, wvb, wob, cosb, sinb, maskp, onesb, out)

    nc.compile()
    return nc


def _emit_kernel(tc, xTb, xT8, wq8, wkb, wvb, wob, cosb, sinb, maskp, onesb, out):
    from contextlib import ExitStack

    nc = tc.nc
    xbr = xTb.rearrange("(hc p) s -> p hc s", p=128)  # [128, 16, B*S]
    x8r = xT8.rearrange("(hc p) s -> p hc s", p=128)
    outr = out.rearrange("s (g e) -> s g e", e=512)  # [B*S, 4, 512]

    with ExitStack() as ctx:
        const = ctx.enter_context(tc.tile_pool(name="const", bufs=1))
        batchp = ctx.enter_context(tc.tile_pool(name="batchp", bufs=2))
        xtp = ctx.enter_context(tc.tile_pool(name="xtp", bufs=2))
        csp = ctx.enter_context(tc.tile_pool(name="csp", bufs=2))
        tmpp = ctx.enter_context(tc.tile_pool(name="tmpp", bufs=2))
        ptp = ctx.enter_context(tc.tile_pool(name="ptp", bufs=15))
        ubp = ctx.enter_context(tc.tile_pool(name="ubp", bufs=2))
        psump = ctx.enter_context(tc.tile_pool(name="psump", bufs=4, space="PSUM"))

        # ---- resident constants ----
        wq_sb = const.tile([128, NJ, 2, DH], FP8)
        wk_sb = const.tile([128, NHC, DH], BF16)
        wv_sb = const.tile([128, NHC, DH], BF16)
        wo_sb = const.tile([128, HPC, HID], BF16)
        mask_sb = const.tile([128, 2, 512], BF16)
        ones_sb = const.tile([128, 1], BF16)
        for j in range(4):
            nc.scalar.dma_start(out=wq_sb[:, j * 2 : j * 2 + 2], in_=wq8[:, j * 2 : j * 2 + 2])
            nc.scalar.dma_start(out=wk_sb[:, j * 4 : j * 4 + 4], in_=wkb[:, j * 4 : j * 4 + 4])
            nc.scalar.dma_start(out=wv_sb[:, j * 4 : j * 4 + 4], in_=wvb[:, j * 4 : j * 4 + 4])
        nc.scalar.dma_start(out=wo_sb[:], in_=wob)
        nc.scalar.dma_start(out=mask_sb[:], in_=maskp)
        nc.scalar.dma_start(out=ones_sb[:], in_=onesb)

        for b in range(B):
            bs = b * S
            qt_sb = batchp.tile([128, HPC, S], BF16, tag="qt")  # Q^T (x32, roped)
            kt_sb = batchp.tile([128, HPC, S], BF16, tag="kt")  # K^T (x32, roped)
            v_sb = batchp.tile([128, NKB, HPC, D], BF16, tag="v")  # V row-blocks
            at_sb = batchp.tile([128, HPC, S], BF16, tag="at")  # attn out (A^T)

            # ---- phase A: QKV projections + RoPE ----
            for st in range(S // TS):
                s0 = st * TS
                g0 = bs + s0
                xtb = xtp.tile([128, NHC, TS], BF16, tag="xb")
                xt8 = xtp.tile([128, NHC, TS], FP8, tag="x8")
                for q4 in range(4):
                    nc.sync.dma_start(
                        out=xtb[:, q4 * 4 : q4 * 4 + 4, :],
                        in_=xbr[:, q4 * 4 : q4 * 4 + 4, g0 : g0 + TS],
                    )
                    nc.sync.dma_start(
                        out=xt8[:, q4 * 4 : q4 * 4 + 4, :],
                        in_=x8r[:, q4 * 4 : q4 * 4 + 4, g0 : g0 + TS],
                    )
                cs = csp.tile([128, TS], BF16, tag="cs")
                nc.sync.dma_start(out=cs[:], in_=cosb[:, g0 : g0 + TS])
                sn = csp.tile([128, TS], BF16, tag="sn")
                nc.sync.dma_start(out=sn[:], in_=sinb[:, g0 : g0 + TS])

                psq = psump.tile([128, 2, TS], F32, tag="ps", name="psq")
                psk = psump.tile([128, 2, TS], F32, tag="ps", name="psk")
                psv = psump.tile([128, 2, 2, HPC, D], F32, tag="ps", name="psv")

                def v_mm(c, sbk):
                    sp, s2 = sbk // 2, sbk % 2
                    nc.tensor.matmul(
                        psv[:, sp, s2],
                        lhsT=xtb[:, c, sbk * 128 : (sbk + 1) * 128],
                        rhs=wv_sb[:, c, :],
                        start=c == 0 and s2 == 0,
                        stop=c == NHC - 1 and s2 == 1,
                        skip_group_check=True,
                    )

                # interleave: 2 Q(DR) + 4 K MMs with 8 V MMs per chunk pair so
                # V's 1:1 LDWEIGHTS load hides under Q/K streaming
                for j in range(NJ):
                    first, last = j == 0, j == NJ - 1
                    for h in range(HPC):
                        nc.tensor.matmul(
                            psq[:, h, :],
                            lhsT=wq_sb[:, j, :, h * D : (h + 1) * D],
                            rhs=xt8[:, 2 * j : 2 * j + 2, :],
                            start=first,
                            stop=last,
                            perf_mode=DR,
                        )
                        v_mm(2 * j, 2 * h)
                    for sub in range(2):
                        c = 2 * j + sub
                        for h in range(HPC):
                            nc.tensor.matmul(
                                psk[:, h, :],
                                lhsT=wk_sb[:, c, h * D : (h + 1) * D],
                                rhs=xtb[:, c, :],
                                start=first and sub == 0,
                                stop=last and sub == 1,
                            )
                            if sub == 0:
                                v_mm(2 * j, 2 * h + 1)
                            else:
                                v_mm(2 * j + 1, 2 * h)
                    v_mm(2 * j + 1, 1)
                    v_mm(2 * j + 1, 3)
                # V: evacuate PSUM -> v_sb (bf16)
                for sp in range(2):
                    kb0 = st * 4 + 2 * sp
                    nc.scalar.copy(v_sb[:, kb0 : kb0 + 2, :, :], psv[:, sp])
                # RoPE on DVE (+gpsimd add): dst = ps*cos + shuffle(ps*sin_sw)
                for ps_t, dst in ((psq, qt_sb), (psk, kt_sb)):
                    for h in range(HPC):
                        ps = ps_t[:, h, :]
                        y = tmpp.tile([128, TS], BF16, tag="y")
                        nc.vector.tensor_mul(y[:], ps, sn[:])
                        ysw = tmpp.tile([128, TS], BF16, tag="ysw")
                        nc.vector.stream_shuffle(ysw[:], y[:], SHUF_MASK)
                        tcs = tmpp.tile([128, TS], BF16, tag="tc")
                        nc.vector.tensor_mul(tcs[:], ps, cs[:])
                        nc.gpsimd.tensor_add(dst[:, h, s0 : s0 + TS], tcs[:], ysw[:])

            # ---- phase B: causal attention ----
            for qt in range(S // TQ):
                q0 = qt * TQ
                npairs = (q0 + TQ) // 256
                # chunk list: (kb0, lo, wc, masked); halves never cross banks
                chunks = []
                for i in range(npairs):
                    kb0 = 2 * i
                    offf = min(max(kb0 * 128 - q0, 0), TQ - 256)
                    diag = kb0 * 128 >= q0
                    for hb in range(2):
                        lo = max(offf, hb * 512)
                        hi = (hb + 1) * 512
                        if lo < hi:
                            chunks.append((kb0, lo, hi - lo, diag and lo == offf))
                half_first = {}
                half_last = {}
                for idx, (kb0, lo, wc, m) in enumerate(chunks):
                    hb = lo // 512
                    if hb not in half_first:
                        half_first[hb] = idx
                    half_last[hb] = idx

                for h in range(HPC):
                    pso = psump.tile([128, TQ], F32, tag="ps", name="pso")
                    pts = []

                    def score_chunk(idx):
                        kb0, lo, wc, masked = chunks[idx]
                        pss = psump.tile([128, 2, 512], F32, tag="ps", name="pss")
                        for t in range(2):
                            nc.tensor.matmul(
                                pss[:, t, 0:wc],
                                lhsT=kt_sb[:, h, (kb0 + t) * 128 : (kb0 + t + 1) * 128],
                                rhs=qt_sb[:, h, q0 + lo : q0 + lo + wc],
                                start=True,
                                stop=True,
                            )
                        if masked:
                            nc.vector.tensor_add(
                                pss[:, :, 0:wc], pss[:, :, 0:wc], mask_sb[:, :, 0:wc]
                            )
                        pt = ptp.tile([128, 2, 512], BF16, tag="pt")
                        nc.scalar.activation(
                            pt[:, :, 0:wc],
                            pss[:, :, 0:wc],
                            func=mybir.ActivationFunctionType.Exp,
                            scale=ALPHA,
                        )
                        pts.append(pt)
                        return idx, pt

                    def av_chunk(idx, pt):
                        kb0, lo, wc, _ = chunks[idx]
                        hb = lo // 512
                        for t in range(2):
                            nc.tensor.matmul(
                                pso[:, lo : lo + wc],
                                lhsT=v_sb[:, kb0 + t, h, :],
                                rhs=pt[:, t, 0:wc],
                                start=half_first[hb] == idx and t == 0,
                                stop=half_last[hb] == idx and t == 1,
                                skip_group_check=True,
                            )

                    pending = None
                    for idx in range(len(chunks)):
                        cur = score_chunk(idx)
                        if pending is not None:
                            av_chunk(*pending)
                        pending = cur
                    av_chunk(*pending)

                    # denominator: batched back-to-back over the retained pt
                    # tiles (keeps the exp->AV pipeline free of the psl bank)
                    psl = psump.tile([1, TQ], F32, tag="ps", name="psl")
                    for idx, (kb0, lo, wc, _m) in enumerate(chunks):
                        hb = lo // 512
                        for t in range(2):
                            nc.tensor.matmul(
                                psl[:, lo : lo + wc],
                                lhsT=ones_sb[:],
                                rhs=pts[idx][:, t, 0:wc],
                                start=half_first[hb] == idx and t == 0,
                                stop=half_last[hb] == idx and t == 1,
                                skip_group_check=True,
                            )
                    rec = tmpp.tile([1, TQ], F32, tag="rec", bufs=1)
                    nc.vector.reciprocal_approx_fast(out=rec[:], in_=psl[:])
                    rb = tmpp.tile([128, TQ], F32, tag="rb", bufs=1)
                    nc.gpsimd.partition_broadcast(rb[:], rec[:])
                    nc.vector.tensor_mul(at_sb[:, h, q0 : q0 + TQ], pso[:], rb[:])

            # ---- phase C: output projection (partial over local heads) ----
            for sb in range(S // 128):
                psu = [
                    psump.tile([128, 2, 512], F32, tag="ps", name=f"psu{ep}")
                    for ep in range(2)
                ]
                for et in range(4):
                    for h in range(HPC):
                        nc.tensor.matmul(
                            psu[et // 2][:, et % 2, :],
                            lhsT=at_sb[:, h, sb * 128 : (sb + 1) * 128],
                            rhs=wo_sb[:, h, et * 512 : (et + 1) * 512],
                            start=h == 0,
                            stop=h == HPC - 1,
                        )
                ub = ubp.tile([128, 4, 512], BF16, tag="ub")
                nc.scalar.copy(ub[:, 0:2, :], psu[0][:])
                nc.vector.tensor_copy(ub[:, 2:4, :], psu[1][:])
                nc.sync.dma_start(
                    out=outr[bs + sb * 128 : bs + (sb + 1) * 128, :, :], in_=ub[:]
                )


def _host_inputs(hidden_states, cos, sin, wq, wk, wv, wo):
    x = np.ascontiguousarray(np.asarray(hidden_states, dtype=np.float32)).reshape(
        B * S, HID
    )
    xT = np.ascontiguousarray(x.T)
    xTb = xT.astype(NP_BF16)
    xT8 = xT.astype(NP_FP8)
    cos = np.asarray(cos, dtype=np.float32).reshape(B * S, D)
    sin = np.asarray(sin, dtype=np.float32).reshape(B * S, D)
    cosb = np.ascontiguousarray(cos.T[PERM]).astype(NP_BF16)  # [128, B*S]
    snfold = SGN[:, None] * sin.T[PERM]
    sinb = np.ascontiguousarray(snfold[PARTNER]).astype(NP_BF16)
    wq = np.asarray(wq, dtype=np.float32)
    wk = np.asarray(wk, dtype=np.float32)
    wv = np.asarray(wv, dtype=np.float32)
    wo = np.asarray(wo, dtype=np.float32)

    jj = np.arange(512)[None, :]
    pp = np.arange(128)[:, None]
    maskp = np.empty((128, 2, 512), np.float32)
    maskp[:, 0, :] = np.where(jj >= pp, 0.0, -1e30)
    maskp[:, 1, :] = np.where(jj >= 128 + pp, 0.0, -1e30)
    maskp = maskp.astype(NP_BF16)
    onesb = np.ones((128, 1), NP_BF16)

    in_maps = []
    for c in range(NCORES):
        sl = slice(c * DH, (c + 1) * DH)
        # wq: [h, dd(permuted), (j t p)] scaled x32 -> fp8 [p, j, t, (h dd)]
        wqp = (SW * wq[sl].reshape(HPC, 128, HID)[:, PERM, :]).reshape(
            HPC, 128, NJ, 2, 128
        )
        wq8 = np.ascontiguousarray(np.transpose(wqp, (4, 2, 3, 0, 1))).reshape(
            128, NJ, 2, DH
        ).astype(NP_FP8)
        # wk: [h, dd(permuted), hid] -> bf16 [p, hc, (h dd)]
        wkp = wk[sl].reshape(HPC, 128, NHC, 128)[:, PERM, :, :]
        wkb = np.ascontiguousarray(np.transpose(wkp, (3, 2, 0, 1))).reshape(
            128, NHC, DH
        ).astype(NP_BF16)
        # wvb[p, c, o] = wv_l[o, c*128+p]
        wvb = np.ascontiguousarray(
            np.transpose(wv[sl].reshape(DH, NHC, 128), (2, 1, 0))
        ).astype(NP_BF16)
        # wob[p, h, e] = wo[e, c0 + h*128 + p]
        wob = np.ascontiguousarray(
            np.transpose(wo[:, sl].reshape(HID, HPC, 128), (2, 1, 0))
        ).astype(NP_BF16)
        in_maps.append(
            {
                "xTb": xTb,
                "xT8": xT8,
                "wq8": wq8,
                "wkb": wkb,
                "wvb": wvb,
                "wob": wob,
                "cosb": cosb,
                "sinb": sinb,
                "maskp": maskp,
                "onesb": onesb,
            }
        )
    return in_maps


def kernel(
    hidden_states,
    cos,
    sin,
    wq,
    wk,
    wv,
    wo,
    position_ids=None,
    _trace=False,
    _tmpdir=None,
):
    global LAST_EXEC_TIME_NS
    if "nc" not in _CACHE:
        _CACHE["nc"] = _build_device_program()
    nc = _CACHE["nc"]
    in_maps = _host_inputs(hidden_states, cos, sin, wq, wk, wv, wo)
    res = run_bass_kernel_spmd(
        nc,
        in_maps,
        list(range(NCORES)),
        trace=_trace,
        tmpdir=_tmpdir,
    )
    LAST_EXEC_TIME_NS = res.exec_time_ns
    total = res.results[0]["out"].astype(np.float64)
    for c in range(1, NCORES):
        total += res.results[c]["out"].astype(np.float64)
    return total.astype(np.float32).reshape(B, S, HID)
